# revision 1
# baseline (speedup 1.0000x reference)
"""LoRA 4-bit linear layer for Trainium2, 8 NeuronCores.

Reference computation (per problem nn_LoRALayer4bit):
    W    = bf16(dequant4bit(q_weight, scales))          # [4096, 4096]
    out  = x @ W.T + 2.0 * ((x @ lora_A.T) @ lora_B.T)  # x: [4, 2048, 4096] bf16

Strategy:
  - Host folds the LoRA low-rank update into the dequantized weight:
        W_eff = bf16(f32(W) + 2.0 * lora_B @ lora_A)
    (differs from the two-path reference by <= 1-2 bf16 ulps on the output).
  - Row-parallel over the 8 cores: each core computes 1024 tokens x full
    4096 out-features (34.4 GFLOP/core).  No collectives; host concatenates.
  - Host pre-transposes each x shard to [K, M] layout and pre-tiles the
    weight so every device DMA is large and contiguous.
  - Device kernel: pure bf16 matmul, K on partitions, accumulating 32
    K-tiles into one PSUM bank per [128 x 512] output tile.
"""

import numpy as np
import ml_dtypes

BF16 = ml_dtypes.bfloat16

IN_F = 4096
OUT_F = 4096
R = 16
SCALING = 2.0
BLK = 64
BATCH = 4
SEQ = 2048
N_CORES = 8

M_TOT = BATCH * SEQ            # 8192 tokens
M_PER = M_TOT // N_CORES       # 1024 tokens per core
KT = IN_F // 128               # 32 contraction tiles
NB = OUT_F // 512              # 8 out-feature blocks
MT = M_PER // 128              # 8 token sub-tiles per core

_CACHE = {}


def _build_nc():
    """Build + compile the single-core SPMD Bass program (cached)."""
    import concourse.bacc as bacc
    import concourse.tile as tile
    from concourse import mybir

    nc = bacc.Bacc(
        "TRN2", target_bir_lowering=False, debug=False, enable_asserts=False
    )

    # xt[k, p, m]    = x_shard.T tile layout (in-feature on partitions)
    # wt[nb, k, p, c] = W_eff.T tiles: [nb*512+c] out-feature, [k*128+p] in-feature
    # out[nb, m, p, c] = out_shard[m*128+p, nb*512+c]
    xt_d = nc.dram_tensor(
        "xt", [KT, 128, M_PER], mybir.dt.bfloat16, kind="ExternalInput"
    )
    wt_d = nc.dram_tensor(
        "wt", [NB, KT, 128, 512], mybir.dt.bfloat16, kind="ExternalInput"
    )
    out_d = nc.dram_tensor(
        "out", [NB, MT, 128, 512], mybir.dt.bfloat16, kind="ExternalOutput"
    )

    with tile.TileContext(nc) as tc:
        with (
            tc.tile_pool(name="xp", bufs=KT) as xp,
            tc.tile_pool(name="wp", bufs=2 * KT) as wp,
            tc.tile_pool(name="op", bufs=4) as op,
            tc.tile_pool(name="pp", bufs=4, space="PSUM") as pp,
        ):
            # x shard stays resident in SBUF for the whole kernel (64KB/part).
            xts = []
            for k in range(KT):
                xtile = xp.tile(
                    [128, M_PER], mybir.dt.bfloat16, name=f"x{k}", tag="xt"
                )
                nc.sync.dma_start(xtile[:], xt_d[k])
                xts.append(xtile)

            for nb in range(NB):
                # Weight block for these 512 out-features; wp has 2 blocks
                # worth of slots so block nb+1 prefetches during block nb.
                wts = []
                for k in range(KT):
                    wtile = wp.tile(
                        [128, 512], mybir.dt.bfloat16, name=f"w{nb}_{k}", tag="wt"
                    )
                    nc.sync.dma_start(wtile[:], wt_d[nb, k])
                    wts.append(wtile)

                for m in range(MT):
                    ps = pp.tile(
                        [128, 512], mybir.dt.float32, name=f"ps{nb}_{m}", tag="ps"
                    )
                    for k in range(KT):
                        nc.tensor.matmul(
                            ps[:],
                            xts[k][:, m * 128 : (m + 1) * 128],
                            wts[k][:],
                            start=(k == 0),
                            stop=(k == KT - 1),
                        )
                    ot = op.tile(
                        [128, 512], mybir.dt.bfloat16, name=f"o{nb}_{m}", tag="ot"
                    )
                    nc.vector.tensor_copy(ot[:], ps[:])
                    nc.sync.dma_start(out_d[nb, m], ot[:])

    nc.compile()
    return nc


def _prep_weights(q_weight, scales, lora_A, lora_B):
    q = np.asarray(q_weight)
    s = np.asarray(scales, dtype=np.float32)
    # Exactly the reference dequant: per-64-block scale, rounded to bf16.
    W = (
        (q.astype(np.float32).reshape(OUT_F, IN_F // BLK, BLK) * s[:, :, None])
        .reshape(OUT_F, IN_F)
        .astype(BF16)
    )
    BA = np.asarray(lora_B, dtype=np.float32) @ np.asarray(lora_A, dtype=np.float32)
    W_eff = (W.astype(np.float32) + SCALING * BA).astype(BF16)
    # [nb, k, p, c] = W_eff[nb*512+c, k*128+p]
    wt = np.ascontiguousarray(
        W_eff.reshape(NB, 512, KT, 128).transpose(0, 2, 3, 1)
    )
    return wt


def kernel(x, q_weight, scales, lora_A, lora_B):
    from concourse.bass_utils import run_bass_kernel_spmd

    if "nc" not in _CACHE:
        _CACHE["nc"] = _build_nc()
    nc = _CACHE["nc"]

    wt = _prep_weights(q_weight, scales, lora_A, lora_B)

    xf = np.ascontiguousarray(np.asarray(x)).reshape(M_TOT, IN_F)
    in_maps = []
    for c in range(N_CORES):
        xs = xf[c * M_PER : (c + 1) * M_PER]          # [1024, 4096]
        xt = np.ascontiguousarray(xs.T).reshape(KT, 128, M_PER)
        in_maps.append({"xt": xt, "wt": wt})

    res = run_bass_kernel_spmd(nc, in_maps, core_ids=list(range(N_CORES)))
    _CACHE["last_results"] = res

    shards = []
    for c in range(N_CORES):
        o = np.asarray(res.results[c]["out"])          # [NB, MT, 128, 512]
        shards.append(o.transpose(1, 2, 0, 3).reshape(M_PER, OUT_F))
    out = np.concatenate(shards, axis=0).reshape(BATCH, SEQ, OUT_F)
    return out.astype(BF16)
